# revision 1
# baseline (speedup 1.0000x reference)
"""Haar wavelet (2x2 block) decomposition kernel for 8 Trainium2 NeuronCores.

Input  x: [16, 32, 512, 512] f32
Output  : [16, 128, 256, 256] f32 = concat([pooled, diffH, diffV, diffD], axis=1)

Sharding: pure data parallel over the batch axis — core i handles batches
[2i, 2i+1] (64 images of 512x512 per core).

Per-image dataflow on one core (all fp32):
  load X [128, 2048]  (partition p = image rows 4p..4p+3, one 1 MiB DMA)
  s = E + O, d = E - O          (row butterfly, DVE, FD=1024)
  po = (s_e + s_o) * 0.25       (column butterfly, DVE + ACT scale)
  dv = (s_e - s_o) * 0.5
  dh = (d_e + d_o) * 0.5
  dd =  d_e - d_o
  4 stores of [128, 512] (each a contiguous 256 KiB output image)

The walrus build available here only accepts ONE sync-wait per instruction
(setupSyncWait: "Too many sync wait commands"), while Tile freely attaches
several.  _split_multi_waits() post-processes the serialized BIR, hoisting
all-but-one wait of every instruction onto single-wait NoOps inserted just
before it (same engine, so per-engine program order is preserved).
"""

import functools

import numpy as np
import orjson

import concourse.bass as bass
import concourse.mybir as mybir
from concourse.tile import TileContext
from concourse.bass_utils import run_bass_kernel_spmd

_N_CORES = 8
_B, _C, _H, _W = 16, 32, 512, 512
_BPC = _B // _N_CORES  # batches per core
_IMGS = _BPC * _C  # images per core
_F32 = mybir.dt.float32


def _split_multi_waits(j: dict) -> dict:
    for fn in j["functions"]:
        for blk in fn["blocks"]:
            out = []
            for ins in blk["instructions"]:
                si = ins.get("sync_info")
                waits = (si or {}).get("on_wait") or []
                if len(waits) > 1:
                    for k, w in enumerate(waits[:-1]):
                        out.append(
                            {
                                "debug": ins.get("debug", 0),
                                "engine": ins["engine"],
                                "ins": [],
                                "outs": [],
                                "name": f"{ins['name']}__w{k}",
                                "opcode": "NoOp",
                                "text_hint": "split_wait",
                                "sync_info": {"on_update": [], "on_wait": [w]},
                            }
                        )
                    si["on_wait"] = [waits[-1]]
                out.append(ins)
            blk["instructions"] = out
    return j


_orig_to_json_bytes = bass.Bass.to_json_bytes


def _patched_to_json_bytes(self):
    j = orjson.loads(_orig_to_json_bytes(self))
    _split_multi_waits(j)
    return orjson.dumps(j)


bass.Bass.to_json_bytes = _patched_to_json_bytes


@functools.lru_cache(maxsize=1)
def _build_nc() -> bass.Bass:
    nc = bass.Bass()
    x = nc.dram_tensor("x", [_IMGS, _H, _W], _F32, kind="ExternalInput")
    y = nc.dram_tensor("y", [4 * _IMGS, _H // 2, _W // 2], _F32, kind="ExternalOutput")

    with TileContext(nc) as tc:
        with tc.tile_pool(name="sbuf", bufs=3) as pool:
            for img in range(_IMGS):
                X = pool.tile([128, 4 * _W], _F32, tag="X")
                nc.sync.dma_start(
                    out=X, in_=x[img].rearrange("(p a) w -> p (a w)", p=128)
                )
                # partition p rows: [4p, 4p+1, 4p+2, 4p+3] = (a, eo) with a=row
                # pair in partition, eo=even/odd row
                Xv = X.rearrange("p (a eo w) -> p eo a w", a=2, eo=2)
                s = pool.tile([128, 2 * _W], _F32, tag="s")
                d = pool.tile([128, 2 * _W], _F32, tag="d")
                sv = s.rearrange("p (a w) -> p a w", a=2)
                dvv = d.rearrange("p (a w) -> p a w", a=2)
                nc.vector.tensor_add(out=sv, in0=Xv[:, 0], in1=Xv[:, 1])
                nc.vector.tensor_sub(out=dvv, in0=Xv[:, 0], in1=Xv[:, 1])
                # column butterfly: split free dim into (x, v) with v = even/odd col
                sr = s.rearrange("p (x v) -> p v x", v=2)
                dr = d.rearrange("p (x v) -> p v x", v=2)
                po = pool.tile([128, _W], _F32, tag="po")
                dh = pool.tile([128, _W], _F32, tag="dh")
                dv = pool.tile([128, _W], _F32, tag="dv")
                dd = pool.tile([128, _W], _F32, tag="dd")
                nc.vector.tensor_add(out=po, in0=sr[:, 0], in1=sr[:, 1])
                nc.vector.tensor_sub(out=dv, in0=sr[:, 0], in1=sr[:, 1])
                nc.vector.tensor_add(out=dh, in0=dr[:, 0], in1=dr[:, 1])
                nc.vector.tensor_sub(out=dd, in0=dr[:, 0], in1=dr[:, 1])
                nc.scalar.mul(po, po, 0.25)
                nc.scalar.mul(dh, dh, 0.5)
                nc.scalar.mul(dv, dv, 0.5)
                b, c = divmod(img, _C)
                for k, t in enumerate((po, dh, dv, dd)):
                    oi = b * (4 * _C) + k * _C + c
                    nc.sync.dma_start(
                        out=y[oi].rearrange("(p a) w -> p (a w)", p=128), in_=t
                    )
    return nc


def _shard_input(x: np.ndarray) -> list[np.ndarray]:
    x = np.ascontiguousarray(np.asarray(x), dtype=np.float32)
    shards = x.reshape(_N_CORES, _IMGS, _H, _W)
    return [shards[i] for i in range(_N_CORES)]


def _unshard_output(per_core: list[np.ndarray]) -> np.ndarray:
    out = np.empty((_B, 4 * _C, _H // 2, _W // 2), np.float32)
    for i in range(_N_CORES):
        out[_BPC * i : _BPC * (i + 1)] = per_core[i].reshape(
            _BPC, 4 * _C, _H // 2, _W // 2
        )
    return out


def kernel(x) -> np.ndarray:
    shards = _shard_input(x)
    in_maps = [{"x": s} for s in shards]
    res = run_bass_kernel_spmd(_build_nc(), in_maps, list(range(_N_CORES)))
    return _unshard_output([res.results[i]["y"] for i in range(_N_CORES)])


# revision 2
# speedup vs baseline: 1.0354x; 1.0354x over previous
"""Haar wavelet (2x2 block) decomposition kernel for 8 Trainium2 NeuronCores.

Input  x: [16, 32, 512, 512] f32
Output  : [16, 128, 256, 256] f32 = concat([pooled, diffH, diffV, diffD], axis=1)

Sharding: pure data parallel over the batch axis — core i handles batches
[2i, 2i+1] (64 images of 512x512 per core).

Per-image dataflow on one core (all fp32):
  load X [128, 2048]  (partition p = image rows 4p..4p+3, one 1 MiB DMA)
  s = E + O, d = E - O          (row butterfly, DVE, FD=1024)
  po = (s_e + s_o) * 0.25       (column butterfly, DVE + ACT scale)
  dv = (s_e - s_o) * 0.5
  dh = (d_e + d_o) * 0.5
  dd =  d_e - d_o
  4 stores of [128, 512] (each a contiguous 256 KiB output image)

The walrus build available here only accepts ONE sync-wait per instruction
(setupSyncWait: "Too many sync wait commands"), while Tile freely attaches
several.  _split_multi_waits() post-processes the serialized BIR, hoisting
all-but-one wait of every instruction onto single-wait NoOps inserted just
before it (same engine, so per-engine program order is preserved).
"""

import functools

import numpy as np
import orjson

import concourse.bass as bass
import concourse.mybir as mybir
from concourse.tile import TileContext
from concourse.bass_utils import run_bass_kernel_spmd

_N_CORES = 8
_B, _C, _H, _W = 16, 32, 512, 512
_BPC = _B // _N_CORES  # batches per core
_IMGS = _BPC * _C  # images per core
_F32 = mybir.dt.float32


def _split_multi_waits(j: dict) -> dict:
    for fn in j["functions"]:
        for blk in fn["blocks"]:
            out = []
            for ins in blk["instructions"]:
                si = ins.get("sync_info")
                waits = (si or {}).get("on_wait") or []
                if len(waits) > 1:
                    for k, w in enumerate(waits[:-1]):
                        out.append(
                            {
                                "debug": ins.get("debug", 0),
                                "engine": ins["engine"],
                                "ins": [],
                                "outs": [],
                                "name": f"{ins['name']}__w{k}",
                                "opcode": "NoOp",
                                "text_hint": "split_wait",
                                "sync_info": {"on_update": [], "on_wait": [w]},
                            }
                        )
                    si["on_wait"] = [waits[-1]]
                out.append(ins)
            blk["instructions"] = out
    return j


_orig_to_json_bytes = bass.Bass.to_json_bytes


def _patched_to_json_bytes(self):
    j = orjson.loads(_orig_to_json_bytes(self))
    _split_multi_waits(j)
    return orjson.dumps(j)


bass.Bass.to_json_bytes = _patched_to_json_bytes


@functools.lru_cache(maxsize=1)
def _build_nc() -> bass.Bass:
    nc = bass.Bass()
    x = nc.dram_tensor("x", [_IMGS, _H, _W], _F32, kind="ExternalInput")
    y = nc.dram_tensor("y", [4 * _IMGS, _H // 2, _W // 2], _F32, kind="ExternalOutput")

    with TileContext(nc) as tc:
        with tc.tile_pool(name="sbuf", bufs=3) as pool:
            # Two consecutive images (same batch b, channels c, c+1) per
            # iteration: one 2 MiB load, and the 4 result stores each cover
            # the two adjacent output channels (one contiguous 512 KiB DMA).
            # Loads go on the SP HWDGE ring, stores on the ACT HWDGE ring so
            # the two rings drive the SDMA pool concurrently.
            for img0 in range(0, _IMGS, 2):
                X = pool.tile([128, 2 * 4 * _W], _F32, tag="X")
                nc.sync.dma_start(
                    out=X.rearrange("p (i aw) -> p i aw", i=2),
                    in_=x[img0 : img0 + 2].rearrange("i (p a) w -> p i (a w)", p=128),
                )
                # per partition p, image i: rows [4p..4p+3] = (a, eo) with
                # a = row-pair within partition, eo = even/odd row
                Xv = X.rearrange("p (i a eo w) -> p eo i a w", i=2, a=2, eo=2)
                s = pool.tile([128, 2 * 2 * _W], _F32, tag="s")
                d = pool.tile([128, 2 * 2 * _W], _F32, tag="d")
                sv = s.rearrange("p (i a w) -> p i a w", i=2, a=2)
                dvv = d.rearrange("p (i a w) -> p i a w", i=2, a=2)
                nc.vector.tensor_add(out=sv, in0=Xv[:, 0], in1=Xv[:, 1])
                nc.vector.tensor_sub(out=dvv, in0=Xv[:, 0], in1=Xv[:, 1])
                # column butterfly: split free dim into (x, v), v = even/odd col
                sr = s.rearrange("p (x v) -> p v x", v=2)
                dr = d.rearrange("p (x v) -> p v x", v=2)
                po = pool.tile([128, 2 * _W], _F32, tag="po")
                dh = pool.tile([128, 2 * _W], _F32, tag="dh")
                dv = pool.tile([128, 2 * _W], _F32, tag="dv")
                dd = pool.tile([128, 2 * _W], _F32, tag="dd")
                nc.vector.tensor_add(out=po, in0=sr[:, 0], in1=sr[:, 1])
                nc.vector.tensor_sub(out=dv, in0=sr[:, 0], in1=sr[:, 1])
                nc.vector.tensor_add(out=dh, in0=dr[:, 0], in1=dr[:, 1])
                nc.vector.tensor_sub(out=dd, in0=dr[:, 0], in1=dr[:, 1])
                nc.scalar.mul(po, po, 0.25)
                nc.scalar.mul(dh, dh, 0.5)
                nc.scalar.mul(dv, dv, 0.5)
                b, c = divmod(img0, _C)
                for k, t in enumerate((po, dh, dv, dd)):
                    oi = b * (4 * _C) + k * _C + c
                    nc.scalar.dma_start(
                        out=y[oi : oi + 2].rearrange("i (p a) w -> p i (a w)", p=128),
                        in_=t.rearrange("p (i aw) -> p i aw", i=2),
                    )
    return nc


def _shard_input(x: np.ndarray) -> list[np.ndarray]:
    x = np.ascontiguousarray(np.asarray(x), dtype=np.float32)
    shards = x.reshape(_N_CORES, _IMGS, _H, _W)
    return [shards[i] for i in range(_N_CORES)]


def _unshard_output(per_core: list[np.ndarray]) -> np.ndarray:
    out = np.empty((_B, 4 * _C, _H // 2, _W // 2), np.float32)
    for i in range(_N_CORES):
        out[_BPC * i : _BPC * (i + 1)] = per_core[i].reshape(
            _BPC, 4 * _C, _H // 2, _W // 2
        )
    return out


def kernel(x) -> np.ndarray:
    shards = _shard_input(x)
    in_maps = [{"x": s} for s in shards]
    res = run_bass_kernel_spmd(_build_nc(), in_maps, list(range(_N_CORES)))
    return _unshard_output([res.results[i]["y"] for i in range(_N_CORES)])
